# revision 1
# baseline (speedup 1.0000x reference)
"""GAT message-passing model on 8 Trainium2 NeuronCores.

Strategy: edges sorted by destination node on the host; nodes split into 8
contiguous ranges balanced by incoming-edge count (one range per core).  Each
core computes the full k/v projection tables (replicated), its local q rows,
then processes its edge shard window-by-window: windows of <=128 contiguous
dst nodes with <=TPW*128 edges, padded to a fixed TPW tiles of 128 edge
slots, so all 8 cores run one identical SPMD instruction stream and differ
only in input data.  Per-edge-tile work: gather k/v rows by src (indirect
DMA), ke = ef@We on PE (+K via identity-matmul PSUM accumulation),
Qe = onehot^T @ q_win on PE, logits via DVE mul + segmented reduce,
w = exp on ACT (broadcast to head width), WV = w*V on DVE, and segment
reduction into the window's node rows via onehot matmuls accumulating in
PSUM.  Window finalize computes sigmoid(relu(agg/denom)@Wd+bd) and scatters
rows to the local output; the host concatenates the 8 node ranges.
"""

import numpy as np
import ml_dtypes

import concourse.bass as bass
import concourse.bacc as bacc
import concourse.mybir as mybir
import concourse.tile as tile

BF16 = ml_dtypes.bfloat16

H, DH = 8, 64
DOUT = H * DH  # 512
N_CORES = 8
TPW = 8  # edge tiles per window
GRP = 4  # tiles gathered per indirect-DMA group


# ----------------------------------------------------------------------------
# Host-side planning
# ----------------------------------------------------------------------------

def make_plan(src, dst, n_nodes, n_cores, tpw):
    E = src.shape[0]
    perm = np.argsort(dst, kind="stable")
    s_src = src[perm]
    s_dst = dst[perm]
    deg = np.bincount(dst, minlength=n_nodes)
    cum = np.concatenate([[0], np.cumsum(deg)])

    cuts = [0]
    for c in range(1, n_cores):
        target = c * E / n_cores
        n = int(np.searchsorted(cum, target))
        n = max(cuts[-1] + 1, min(n, n_nodes - (n_cores - c)))
        cuts.append(n)
    cuts.append(n_nodes)

    cores = []
    for c in range(n_cores):
        nlo, nhi = cuts[c], cuts[c + 1]
        wins = []
        n = nlo
        while n < nhi:
            n2 = n
            edges = 0
            while n2 < nhi and (n2 - n) < 128:
                if edges + deg[n2] > tpw * 128:
                    break
                edges += deg[n2]
                n2 += 1
            assert n2 > n, f"node {n} degree {deg[n]} > {tpw*128}"
            wins.append((n, n2))
            n = n2
        cores.append(dict(nlo=nlo, nhi=nhi, wins=wins))

    NWIN = max(len(c["wins"]) for c in cores)
    QCHUNK = max((c["nhi"] - c["nlo"] + 127) // 128 for c in cores)
    return dict(cores=cores, NWIN=NWIN, QCHUNK=QCHUNK, TPW=tpw,
                s_src=s_src, s_dst=s_dst, perm=perm, cum=cum)


def make_core_inputs(plan, core_idx, ef_sorted, nfT_bf16):
    tpw = plan["TPW"]
    NWIN = plan["NWIN"]
    QCHUNK = plan["QCHUNK"]
    core = plan["cores"][core_idx]
    s_src, cum = plan["s_src"], plan["cum"]
    DE = ef_sorted.shape[1]
    nlo = core["nlo"]
    L = core["nhi"] - nlo
    trash = QCHUNK * 128

    gsrc = np.zeros((NWIN, 128, tpw), np.int32)
    dstloc = np.full((NWIN, 128, tpw), 255.0, BF16)
    efT = np.zeros((NWIN * tpw // GRP, 64, 128 * GRP), BF16)
    dstlocT = np.full((NWIN * tpw // GRP, 128, 128 * GRP), 255.0, BF16)
    wnodes = np.full((NWIN, 128, 1), trash, np.int32)

    for w, (wn_lo, wn_hi) in enumerate(core["wins"]):
        e0, e1 = cum[wn_lo], cum[wn_hi]
        cnt = e1 - e0
        wn = np.arange(wn_lo, wn_hi) - nlo
        wnodes[w, : len(wn), 0] = wn
        sl = np.arange(cnt)
        t_idx = sl // 128
        p_idx = sl % 128
        gsrc[w, p_idx, t_idx] = s_src[e0:e1]
        dl = (plan["s_dst"][e0:e1] - wn_lo).astype(BF16)
        dstloc[w, p_idx, t_idx] = dl
        for t in range(tpw):
            m = t_idx == t
            if not m.any():
                continue
            grp = (w * tpw + t) // GRP
            j = t % GRP
            efT[grp, :DE, j * 128 + p_idx[m]] = ef_sorted[e0:e1][m].astype(BF16)
            dstlocT[grp, :, j * 128 + p_idx[m]] = np.tile(dl[m][:, None], (1, 128))

    nfT_l = np.zeros((QCHUNK, 128, 256), BF16)
    nhi = core["nhi"]
    for i in range(QCHUNK):
        a = nlo + i * 128
        b = min(a + 128, nhi)
        if b > a:
            blk = nfT_bf16[:, a:b]
            nfT_l[i, :, 0 : b - a] = blk[:128]
            nfT_l[i, :, 128 : 128 + b - a] = blk[128:256]
    return dict(gsrc=gsrc, dstloc=dstloc, efT=efT, dstlocT=dstlocT,
                wnodes=wnodes, nfT_l=nfT_l, L=L, nlo=nlo)


def make_global_inputs(nf, Wq, Wk, Wv, We, Wd):
    N, DIN = nf.shape
    nfT = nf.T.astype(BF16)
    NCHUNK = (N + 127) // 128
    nfT_g = np.zeros((NCHUNK, 128, 256), BF16)
    for i in range(NCHUNK):
        a, b = i * 128, min(i * 128 + 128, N)
        nfT_g[i, :, 0 : b - a] = nfT[:128, a:b]
        nfT_g[i, :, 128 : 128 + b - a] = nfT[128:256, a:b]
    scale = 1.0 / np.sqrt(DH)

    def pack_w(W):
        return np.concatenate([W[:128], W[128:256]], axis=1).astype(BF16)

    we_p = np.zeros((64, DOUT), BF16)
    we_p[: We.shape[0]] = We.astype(BF16)
    return dict(
        nfT_g=nfT_g,
        wq=pack_w(Wq * scale),
        wk=pack_w(Wk),
        wv=pack_w(Wv),
        we=we_p,
        wdrow=np.tile(Wd.reshape(1, DOUT), (128, 1)).astype(BF16),
        ident=np.eye(128, dtype=BF16),
        iota_rows=np.tile(np.arange(128, dtype=BF16)[None, :], (128, 1)),
        iota_col=np.arange(128, dtype=BF16).reshape(128, 1),
        nfT=nfT,
        NCHUNK=NCHUNK,
        N=N,
    )


# ----------------------------------------------------------------------------
# Device kernel emission (identical instruction stream on every core)
# ----------------------------------------------------------------------------

def build_nc(N, NCHUNK, NWIN, tpw, QCHUNK, bd0, dbg=False):
    dt = mybir.dt
    bf16, f32, i32 = dt.bfloat16, dt.float32, dt.int32
    NGRP = NWIN * tpw // GRP
    YROWS = QCHUNK * 128 + 128

    nc = bacc.Bacc("TRN2", target_bir_lowering=False, debug=False)

    t_nfT_g = nc.dram_tensor("nfT_g", [NCHUNK, 128, 256], bf16, kind="ExternalInput")
    t_nfT_l = nc.dram_tensor("nfT_l", [QCHUNK, 128, 256], bf16, kind="ExternalInput")
    t_wq = nc.dram_tensor("wq", [128, 2 * DOUT], bf16, kind="ExternalInput")
    t_wk = nc.dram_tensor("wk", [128, 2 * DOUT], bf16, kind="ExternalInput")
    t_wv = nc.dram_tensor("wv", [128, 2 * DOUT], bf16, kind="ExternalInput")
    t_we = nc.dram_tensor("we", [64, DOUT], bf16, kind="ExternalInput")
    t_wdrow = nc.dram_tensor("wdrow", [128, DOUT], bf16, kind="ExternalInput")
    t_ident = nc.dram_tensor("ident", [128, 128], bf16, kind="ExternalInput")
    t_iota_rows = nc.dram_tensor("iota_rows", [128, 128], bf16, kind="ExternalInput")
    t_iota_col = nc.dram_tensor("iota_col", [128, 1], bf16, kind="ExternalInput")
    t_gsrc = nc.dram_tensor("gsrc", [NWIN, 128, tpw], i32, kind="ExternalInput")
    t_dstloc = nc.dram_tensor("dstloc", [NWIN, 128, tpw], bf16, kind="ExternalInput")
    t_dstlocT = nc.dram_tensor("dstlocT", [NGRP, 128, 128 * GRP], bf16, kind="ExternalInput")
    t_efT = nc.dram_tensor("efT", [NGRP, 64, 128 * GRP], bf16, kind="ExternalInput")
    t_wnodes = nc.dram_tensor("wnodes", [NWIN, 128, 1], i32, kind="ExternalInput")

    t_y = nc.dram_tensor("y_out", [YROWS, 1], f32, kind="ExternalOutput")
    t_dbg = {}
    if dbg:
        for nm, shp in [("d_k4", [128, 2 * DOUT]),
                        ("d_ohe", [128, 128]), ("d_ohT", [128, 128]),
                        ("d_kke", [128, DOUT]), ("d_qe", [128, DOUT]),
                        ("d_logits", [128, H]), ("d_wbig", [128, DOUT]),
                        ("d_qwin", [128, DOUT]),
                        ("d_den", [128, H]), ("d_xnorm", [128, DOUT])]:
            t_dbg[nm] = nc.dram_tensor(nm, shp, f32, kind="ExternalOutput")

    def dump(nm, ap):
        if dbg:
            nc.gpsimd.dma_start(out=t_dbg[nm][: ap.shape[0]], in_=ap)

    t_kv = nc.dram_tensor("kv_table", [N, 2 * DOUT], bf16, kind="Internal")
    t_qt = nc.dram_tensor("q_table", [YROWS, DOUT], bf16, kind="Internal")

    with tile.TileContext(nc, pool_alloc_mode="queue") as tc:
        with tc.tile_pool(name="wpool", bufs=1) as wpool:
            wq_sb = wpool.tile([128, 2 * DOUT], bf16)
            nc.sync.dma_start(out=wq_sb[:], in_=t_wq[:])
            wk_sb = wpool.tile([128, 2 * DOUT], bf16)
            nc.sync.dma_start(out=wk_sb[:], in_=t_wk[:])
            wv_sb = wpool.tile([128, 2 * DOUT], bf16)
            nc.sync.dma_start(out=wv_sb[:], in_=t_wv[:])
            we_sb = wpool.tile([64, DOUT], bf16)
            nc.sync.dma_start(out=we_sb[:], in_=t_we[:])
            wdrow_sb = wpool.tile([128, DOUT], bf16)
            nc.sync.dma_start(out=wdrow_sb[:], in_=t_wdrow[:])
            ident_sb = wpool.tile([128, 128], bf16)
            nc.sync.dma_start(out=ident_sb[:], in_=t_ident[:])
            iota_rows_sb = wpool.tile([128, 128], bf16)
            nc.sync.dma_start(out=iota_rows_sb[:], in_=t_iota_rows[:])
            iota_col_sb = wpool.tile([128, 1], bf16)
            nc.sync.dma_start(out=iota_col_sb[:], in_=t_iota_col[:])

            # ---------------- phase 1: k/v tables (all nodes) ----------------
            table_writes = []
            with tc.tile_pool(name="p1", bufs=8) as p1, \
                 tc.tile_pool(name="p1ps", bufs=2, space="PSUM") as p1ps:
                for i in range(NCHUNK):
                    m = min(128, N - i * 128)
                    xt = p1.tile([128, 256], bf16, tag="xt")
                    nc.sync.dma_start(out=xt[:], in_=t_nfT_g[i])
                    ps_kv = p1ps.tile([128, 2 * DOUT], f32, tag="k")
                    for c in range(2):
                        nc.tensor.matmul(ps_kv[:m, :DOUT], xt[:, c * 128 : c * 128 + m], wk_sb[:, c * DOUT : (c + 1) * DOUT],
                                         start=(c == 0), stop=(c == 1))
                        nc.tensor.matmul(ps_kv[:m, DOUT:], xt[:, c * 128 : c * 128 + m], wv_sb[:, c * DOUT : (c + 1) * DOUT],
                                         start=(c == 0), stop=(c == 1))
                    kv_sb = p1.tile([128, 2 * DOUT], bf16, tag="ksb")
                    nc.vector.tensor_copy(kv_sb[:m, :DOUT], ps_kv[:m, :DOUT])
                    nc.scalar.copy(kv_sb[:m, DOUT:], ps_kv[:m, DOUT:])
                    table_writes.append(nc.sync.dma_start(
                        out=t_kv[i * 128 : i * 128 + m, :], in_=kv_sb[:m]))
                # ---------------- phase 1b: q table (local nodes) ----------
                for i in range(QCHUNK):
                    xt = p1.tile([128, 256], bf16, tag="xt")
                    nc.sync.dma_start(out=xt[:], in_=t_nfT_l[i])
                    ps_q = p1ps.tile([128, DOUT], f32, tag="k")
                    for c in range(2):
                        nc.tensor.matmul(ps_q[:], xt[:, c * 128 : (c + 1) * 128], wq_sb[:, c * DOUT : (c + 1) * DOUT],
                                         start=(c == 0), stop=(c == 1))
                    q_sb = p1.tile([128, DOUT], bf16, tag="ksb")
                    nc.vector.tensor_copy(q_sb[:], ps_q[:])
                    table_writes.append(nc.sync.dma_start(
                        out=t_qt[i * 128 : (i + 1) * 128, :], in_=q_sb[:]))
                # zero the trash pad block (gathered by padded window slots)
                zq = p1.tile([128, DOUT], bf16, tag="ksb")
                nc.gpsimd.memset(zq[:], 0)
                table_writes.append(nc.sync.dma_start(
                    out=t_qt[QCHUNK * 128 :, :], in_=zq[:]))

            # Pool-side fence: the gathers are the only table readers and all
            # issue from the Pool sequencer.  Route the fan-in of table-write
            # completion waits into one Pool compute op so no gather DMA ends
            # up with more waits than the DMA lowering allows.
            fence_tile = wpool.tile([1, 4], mybir.dt.int32)
            fence = nc.gpsimd.memset(fence_tile[:], 0)
            for wdma in table_writes:
                tile.add_dep_helper(fence.ins, wdma.ins, sync=True,
                                    reason="table fence")

            # ---------------- phase 2: edge phase ----------------
            with tc.tile_pool(name="p2", bufs=4) as p2, \
                 tc.tile_pool(name="p2s", bufs=8) as p2s, \
                 tc.tile_pool(name="p2w", bufs=2) as p2w, \
                 tc.tile_pool(name="psA", bufs=2, space="PSUM") as psA, \
                 tc.tile_pool(name="psB", bufs=2, space="PSUM") as psB:
                for w in range(NWIN):
                    widx = p2w.tile([128, 1], i32, tag="widx")
                    nc.sync.dma_start(out=widx[:], in_=t_wnodes[w])
                    widx_y = p2w.tile([128, 1], i32, tag="widx_y")
                    nc.sync.dma_start(out=widx_y[:], in_=t_wnodes[w])
                    qwin = p2w.tile([128, DOUT], bf16, tag="qwin")
                    nc.gpsimd.indirect_dma_start(
                        out=qwin[:], out_offset=None, in_=t_qt[:],
                        in_offset=bass.IndirectOffsetOnAxis(ap=widx[:, :1], axis=0))
                    dloc = p2w.tile([128, tpw], bf16, tag="dloc")
                    nc.sync.dma_start(out=dloc[:], in_=t_dstloc[w])
                    idx_w = p2w.tile([128, tpw], i32, tag="idx_w")
                    nc.sync.dma_start(out=idx_w[:], in_=t_gsrc[w])
                    agg = psA.tile([128, DOUT], f32, tag="agg")
                    den = psA.tile([128, H], f32, tag="den")
                    for g in range(tpw // GRP):
                        grp = w * (tpw // GRP) + g
                        efT4 = p2.tile([64, 128 * GRP], bf16, tag="efT4")
                        nc.sync.dma_start(out=efT4[:], in_=t_efT[grp])
                        dT4 = p2.tile([128, 128 * GRP], bf16, tag="dT4", bufs=8)
                        nc.sync.dma_start(out=dT4[:], in_=t_dstlocT[grp])
                        for j in range(GRP):
                            t = g * GRP + j
                            kvg = p2.tile([128, 2 * DOUT], bf16, tag="kvg")
                            nc.gpsimd.indirect_dma_start(
                                out=kvg[:], out_offset=None, in_=t_kv[:],
                                in_offset=bass.IndirectOffsetOnAxis(ap=idx_w[:, t : t + 1], axis=0))
                            k_j = kvg[:, :DOUT]
                            v_j = kvg[:, DOUT:]
                            ps_ke = psB.tile([128, DOUT], f32, tag="ke")
                            nc.tensor.matmul(ps_ke[:], efT4[:, j * 128 : (j + 1) * 128],
                                             we_sb[:], start=True, stop=False)
                            nc.tensor.matmul(ps_ke[:], ident_sb[:], k_j,
                                             start=False, stop=True)
                            kke = p2.tile([128, DOUT], bf16, tag="kke")
                            nc.scalar.copy(kke[:], ps_ke[:])
                            oh_e = p2.tile([128, 128], bf16, tag="oh_e")
                            nc.vector.tensor_tensor(
                                oh_e[:], dloc[:, t : t + 1].to_broadcast([128, 128]),
                                iota_rows_sb[:], mybir.AluOpType.is_equal)
                            oh_T = p2.tile([128, 128], bf16, tag="oh_T")
                            nc.vector.tensor_tensor(
                                oh_T[:], iota_col_sb[:].to_broadcast([128, 128]),
                                dT4[:, j * 128 : (j + 1) * 128],
                                mybir.AluOpType.is_equal)
                            ps_qe = psB.tile([128, DOUT], f32, tag="qe")
                            nc.tensor.matmul(ps_qe[:], oh_T[:], qwin[:],
                                             start=True, stop=True)
                            if dbg and w == 0 and t == 0:
                                dump("d_k4", kvg[:])
                                dump("d_ohe", oh_e[:]); dump("d_ohT", oh_T[:])
                                dump("d_kke", kke[:]); dump("d_qwin", qwin[:])
                            qe = p2.tile([128, DOUT], bf16, tag="qe_sb")
                            nc.scalar.copy(qe[:], ps_qe[:])
                            prod = p2.tile([128, DOUT], bf16, tag="prod")
                            nc.vector.tensor_tensor(prod[:], qe[:], kke[:],
                                                    mybir.AluOpType.mult)
                            logits = p2.tile([128, H], f32, tag="logits")
                            nc.vector.tensor_reduce(
                                logits[:], prod[:].rearrange("p (h d) -> p h d", h=H),
                                mybir.AxisListType.X, mybir.AluOpType.add)
                            wbig = p2.tile([128, H], bf16, tag="wbig")
                            nc.scalar.activation(wbig[:], logits[:],
                                mybir.ActivationFunctionType.Exp)
                            if dbg and w == 0 and t == 0:
                                dump("d_qe", qe[:]); dump("d_logits", logits[:])
                                dump("d_wbig", wbig[:])
                            wv_t = p2.tile([128, DOUT], bf16, tag="wv")
                            nc.vector.tensor_tensor(
                                wv_t[:].rearrange("p (h d) -> p h d", h=H),
                                wbig[:, :, None].to_broadcast([128, H, DH]),
                                v_j.rearrange("p (h d) -> p h d", h=H),
                                mybir.AluOpType.mult)
                            nc.tensor.matmul(agg[:], oh_e[:], wv_t[:],
                                             start=(t == 0), stop=(t == tpw - 1))
                            nc.tensor.matmul(den[:], oh_e[:], wbig[:],
                                             start=(t == 0), stop=(t == tpw - 1))
                    den_sb = p2w.tile([128, H], f32, tag="den_sb")
                    nc.vector.tensor_scalar_add(den_sb[:], den[:], 1e-9)
                    recip = p2w.tile([128, H], f32, tag="recip")
                    nc.vector.reciprocal(recip[:], den_sb[:])
                    xnorm = p2w.tile([128, DOUT], bf16, tag="xnorm")
                    nc.vector.tensor_tensor(
                        xnorm[:].rearrange("p (h d) -> p h d", h=H),
                        agg[:].rearrange("p (h d) -> p h d", h=H),
                        recip[:, :, None].to_broadcast([128, H, DH]),
                        mybir.AluOpType.mult)
                    if dbg and w == 0:
                        dump("d_den", den_sb[:])
                        dump("d_xnorm", xnorm[:])
                    scr = p2w.tile([128, DOUT], bf16, tag="scr")
                    ypre = p2w.tile([128, 1], f32, tag="ypre")
                    nc.vector.scalar_tensor_tensor(
                        out=scr[:], in0=xnorm[:], scalar=0.0, in1=wdrow_sb[:],
                        op0=mybir.AluOpType.max, op1=mybir.AluOpType.mult,
                        accum_out=ypre[:])
                    y_sb = p2w.tile([128, 1], f32, tag="y_sb")
                    nc.scalar.activation(y_sb[:], ypre[:],
                                         mybir.ActivationFunctionType.Sigmoid,
                                         bias=float(bd0))
                    nc.gpsimd.indirect_dma_start(
                        out=t_y[:], out_offset=bass.IndirectOffsetOnAxis(
                            ap=widx_y[:, :1], axis=0),
                        in_=y_sb[:], in_offset=None)
    nc.compile()
    return nc


# ----------------------------------------------------------------------------
# Entry point
# ----------------------------------------------------------------------------

LAST_RESULTS = None  # BassKernelResults of the most recent run (for profiling)
LAST_NC = None


def kernel(node_features, edge_features, Wq, Wk, Wv, We, Wd, bd, src, dst,
           trace=False):
    from concourse.bass_utils import run_bass_kernel_spmd

    nf = np.asarray(node_features, dtype=np.float32)
    ef = np.asarray(edge_features, dtype=np.float32)
    src = np.asarray(src, dtype=np.int32)
    dst = np.asarray(dst, dtype=np.int32)
    Wq = np.asarray(Wq, np.float32)
    Wk = np.asarray(Wk, np.float32)
    Wv = np.asarray(Wv, np.float32)
    We = np.asarray(We, np.float32)
    Wd = np.asarray(Wd, np.float32)
    bd = np.asarray(bd, np.float32)
    N = nf.shape[0]

    plan = make_plan(src, dst, N, N_CORES, TPW)
    gin = make_global_inputs(nf, Wq, Wk, Wv, We, Wd)
    ef_sorted = ef[plan["perm"]]

    nc = build_nc(N=gin["N"], NCHUNK=gin["NCHUNK"], NWIN=plan["NWIN"],
                  tpw=TPW, QCHUNK=plan["QCHUNK"], bd0=float(bd.ravel()[0]))

    shared = {k: gin[k] for k in ("nfT_g", "wq", "wk", "wv", "we", "wdrow",
                                  "ident", "iota_rows", "iota_col")}
    in_maps = []
    core_meta = []
    for c in range(N_CORES):
        cin = make_core_inputs(plan, c, ef_sorted, gin["nfT"])
        m = dict(shared)
        for k in ("nfT_l", "gsrc", "dstloc", "dstlocT", "efT", "wnodes"):
            m[k] = cin[k]
        in_maps.append(m)
        core_meta.append((cin["nlo"], cin["L"]))

    res = run_bass_kernel_spmd(nc, in_maps, core_ids=list(range(N_CORES)),
                               trace=trace)
    global LAST_RESULTS, LAST_NC
    LAST_RESULTS = res
    LAST_NC = nc

    y = np.zeros((N, 1), np.float32)
    for c, (nlo, L) in enumerate(core_meta):
        y[nlo : nlo + L, 0] = res.results[c]["y_out"][:L, 0]
    return y



# revision 23
# speedup vs baseline: 1.9879x; 1.9879x over previous
"""GAT message-passing model on 8 Trainium2 NeuronCores.

Strategy: edges sorted by destination on the host; nodes split into 8
contiguous ranges balanced by incoming-edge count (one per core).  Windows of
<=128 contiguous dst nodes with <=TPW*128 edges, padded to TPW tiles of 128
edge slots so all 8 cores run one identical SPMD instruction stream.

The host pre-gathers (pure layout work, no arithmetic) the transposed node
features for every edge slot and window-node block, plus both one-hot
orientations of the edge->node incidence.  The device kernel is then a single
homogeneous window loop with no tables, no fences and no indirect gathers
except the final y scatter:

  per window:  qwin = nfTwin^T @ Wq (PE) -> SBUF
  per tile:    kkeT[hd,e] = Wk^T nfT_e + We^T efT      (PE, fp8 DoubleRow)
               qeT[hd,e]  = qwin^T @ oh_T              (PE)
               prodT      = qeT * kkeT                 (DVE, one PSUM read)
               logitsT    = sel^T @ prodT              (PE head reduce)
               wT         = exp(logitsT)               (ACT)
               w          = wT^T via ident8            (PE)
               v[e,hd]    = nfT_e^T @ Wv               (PE, fp8 DoubleRow)
               wv_ext     = [w*v | w]                  (DVE + ACT)
               agg       += oh_e^T @ wv_ext            (PE segment-sum)
  finalize:    den=agg[:,512:]+eps; recip (DVE); scr=relu(agg) (ACT);
               u=scr*wd (DVE 2x); z=sum(u*recip) (DVE TTR);
               y=1/(1+exp(-(z+bd))) (ACT exp + DVE); scatter y (POOL).
"""

import numpy as np
import ml_dtypes

import concourse.bass as bass
import concourse.bacc as bacc
import concourse.mybir as mybir
import concourse.tile as tile

BF16 = ml_dtypes.bfloat16
FP8 = ml_dtypes.float8_e4m3

H, DH = 8, 64
DIN, DE = 256, 64
DOUT = H * DH  # 512
N_CORES = 8
TPW = 8  # edge tiles per window
K_FP8 = False
V_FP8 = False


# ----------------------------------------------------------------------------
# Host-side planning (layout only -- no arithmetic on features/weights)
# ----------------------------------------------------------------------------

def make_plan(src, dst, n_nodes, n_cores, tpw):
    E = src.shape[0]
    perm = np.argsort(dst, kind="stable")
    s_src = src[perm]
    s_dst = dst[perm]
    deg = np.bincount(dst, minlength=n_nodes)
    cum = np.concatenate([[0], np.cumsum(deg)])

    cuts = [0]
    for c in range(1, n_cores):
        target = c * E / n_cores
        n = int(np.searchsorted(cum, target))
        n = max(cuts[-1] + 1, min(n, n_nodes - (n_cores - c)))
        cuts.append(n)
    cuts.append(n_nodes)

    cores = []
    for c in range(n_cores):
        nlo, nhi = cuts[c], cuts[c + 1]
        wins = []
        n = nlo
        while n < nhi:
            n2 = n
            edges = 0
            while n2 < nhi and (n2 - n) < 128:
                if edges + deg[n2] > tpw * 128:
                    break
                edges += deg[n2]
                n2 += 1
            assert n2 > n, f"node {n} degree {deg[n]} > {tpw*128}"
            wins.append((n, n2))
            n = n2
        cores.append(dict(nlo=nlo, nhi=nhi, wins=wins))

    NWIN = max(len(c["wins"]) for c in cores)
    LMAX = max(c["nhi"] - c["nlo"] for c in cores)
    return dict(cores=cores, NWIN=NWIN, LMAX=LMAX, TPW=tpw,
                s_src=s_src, s_dst=s_dst, perm=perm, cum=cum)


def make_core_inputs(plan, core_idx, nf_bf, nf_e_dt, ef_sorted_bf):
    """Per-core pre-gathered tensors.  nf_e_dt: nf in the k-path dtype."""
    tpw = plan["TPW"]
    NWIN = plan["NWIN"]
    LMAX = plan["LMAX"]
    core = plan["cores"][core_idx]
    s_src, s_dst, cum = plan["s_src"], plan["s_dst"], plan["cum"]
    nlo = core["nlo"]
    L = core["nhi"] - nlo
    trash = LMAX

    nfe = np.zeros((NWIN, 128, tpw, 2, 128), nf_e_dt.dtype)
    ohT = np.zeros((NWIN, 128, tpw * 128), BF16)
    ohE = np.zeros((NWIN, 128, tpw * 128), BF16)
    efT = np.zeros((NWIN, 64, tpw * 128), BF16)
    nfw = np.zeros((NWIN, 128, 2, 128), BF16)
    wnodes = np.full((NWIN, 128, 1), trash, np.int32)

    for w, (wn_lo, wn_hi) in enumerate(core["wins"]):
        e0, e1 = cum[wn_lo], cum[wn_hi]
        cnt = e1 - e0
        Lw = wn_hi - wn_lo
        wnodes[w, :Lw, 0] = np.arange(wn_lo, wn_hi) - nlo
        sl = np.arange(cnt)
        t_idx = sl // 128
        p_idx = sl % 128
        dl = s_dst[e0:e1] - wn_lo
        # transposed gathered node features: nfe[w, p, t, i, e] = nf[src, i*128+p]
        blk = nf_e_dt[s_src[e0:e1]].reshape(cnt, 2, 128)  # [slot, i, p]
        nfe[w][:, t_idx, :, p_idx] = blk.transpose(0, 2, 1)
        # one-hots
        ohT[w][dl, t_idx * 128 + p_idx] = 1.0
        ohE[w][p_idx, t_idx * 128 + dl] = 1.0
        # transposed edge features
        efT[w][:, t_idx * 128 + p_idx] = ef_sorted_bf[e0:e1].T
        # transposed window-node features for q
        nblk = nf_bf[wn_lo:wn_hi].reshape(Lw, 2, 128)  # [nl, i, p]
        nfw[w][:, :, :Lw] = nblk.transpose(2, 1, 0)
    out = dict(nfe=nfe.reshape(NWIN, 128, tpw * 256),
               ohT=ohT, ohE=ohE, efT=efT,
               nfw=nfw.reshape(NWIN, 128, 256),
               wnodes=wnodes, L=L, nlo=nlo)
    if K_FP8 != V_FP8:
        out["nfe2"] = out["nfe"].astype(FP8 if V_FP8 else BF16)
    return out


def make_global_inputs(Wq, Wk, Wv, We, Wd):
    scale = 1.0 / np.sqrt(DH)
    # DoubleRow K-packing: w3[p, i, n] = W[i*128+p, n]
    wk3 = np.zeros((128, 2, DOUT), FP8 if K_FP8 else BF16)
    wv3 = np.zeros((128, 2, DOUT), FP8 if V_FP8 else BF16)
    for i in range(2):
        wk3[:, i, :] = Wk[i * 128:(i + 1) * 128].astype(wk3.dtype)
        wv3[:, i, :] = Wv[i * 128:(i + 1) * 128].astype(wv3.dtype)
    # wq[p, c*512+n] = (Wq*scale)[c*128+p, n]
    wq = np.concatenate([(Wq * scale)[:128], (Wq * scale)[128:256]],
                        axis=1).astype(BF16)
    we = We.astype(BF16)                       # [64, 512]
    wdrow = np.tile(Wd.reshape(1, DOUT), (128, 1)).astype(BF16)
    return dict(wk3=wk3.reshape(128, 2 * DOUT), wv3=wv3.reshape(128, 2 * DOUT),
                wq=wq, we=we, wdrow=wdrow)


# ----------------------------------------------------------------------------
# Device kernel emission (identical instruction stream on every core)
# ----------------------------------------------------------------------------

def build_nc(NWIN, tpw, LMAX, bd0, k_fp8, v_fp8, dbg=False):
    dt = mybir.dt
    bf16, f32, i32 = dt.bfloat16, dt.float32, dt.int32
    kdt = dt.float8e4 if k_fp8 else dt.bfloat16
    vdt = dt.float8e4 if v_fp8 else dt.bfloat16
    YROWS = LMAX + 128

    nc = bacc.Bacc("TRN2", target_bir_lowering=False, debug=False)

    t_nfe = nc.dram_tensor("nfe", [NWIN, 128, tpw * 256], kdt, kind="ExternalInput")
    t_nfe2 = (nc.dram_tensor("nfe2", [NWIN, 128, tpw * 256], vdt,
                             kind="ExternalInput") if k_fp8 != v_fp8 else None)
    t_ohT = nc.dram_tensor("ohT", [NWIN, 128, tpw * 128], bf16, kind="ExternalInput")
    t_ohE = nc.dram_tensor("ohE", [NWIN, 128, tpw * 128], bf16, kind="ExternalInput")
    t_efT = nc.dram_tensor("efT", [NWIN, 64, tpw * 128], bf16, kind="ExternalInput")
    t_nfw = nc.dram_tensor("nfw", [NWIN, 128, 256], bf16, kind="ExternalInput")
    t_wk3 = nc.dram_tensor("wk3", [128, 2 * DOUT], kdt, kind="ExternalInput")
    t_wv3 = nc.dram_tensor("wv3", [128, 2 * DOUT], vdt, kind="ExternalInput")
    t_wq = nc.dram_tensor("wq", [128, 2 * DOUT], bf16, kind="ExternalInput")
    t_we = nc.dram_tensor("we", [64, DOUT], bf16, kind="ExternalInput")
    t_wdrow = nc.dram_tensor("wdrow", [128, DOUT], bf16, kind="ExternalInput")

    t_y = nc.dram_tensor("y_out", [NWIN, 128, 1], f32, kind="ExternalOutput")
    t_dbg = {}
    if dbg:
        for nm, shp in [("d_kke", [128, 512]), ("d_qe", [128, 512]),
                        ("d_prod", [128, 512]), ("d_logit", [128, 8]),
                        ("d_wv", [128, 520]), ("d_qwin", [128, 512]),
                        ("d_den", [128, 8]), ("d_scr", [128, 512]),
                        ("d_z", [128, 1])]:
            t_dbg[nm] = nc.dram_tensor(nm, shp, f32, kind="ExternalOutput")

    def dump(nm, ap):
        if dbg:
            nc.gpsimd.dma_start(out=t_dbg[nm][: ap.shape[0]], in_=ap)

    MM = mybir.MatmulPerfMode.DoubleRow

    with tile.TileContext(nc, pool_alloc_mode="queue") as tc:
        with tc.tile_pool(name="wt", bufs=1) as wt, \
             tc.tile_pool(name="win", bufs=4) as win, \
             tc.tile_pool(name="tb", bufs=5) as tb, \
             tc.tile_pool(name="psKQ", bufs=2, space="PSUM") as psKQ, \
             tc.tile_pool(name="psV", bufs=2, space="PSUM") as psV, \
             tc.tile_pool(name="psA", bufs=1, space="PSUM") as psA:
            wk3 = wt.tile([128, 2 * DOUT], kdt)
            nc.sync.dma_start(out=wk3[:], in_=t_wk3[:])
            wv3 = wt.tile([128, 2 * DOUT], vdt)
            nc.sync.dma_start(out=wv3[:], in_=t_wv3[:])
            wq = wt.tile([128, 2 * DOUT], bf16)
            nc.sync.dma_start(out=wq[:], in_=t_wq[:])
            we = wt.tile([64, DOUT], bf16)
            nc.sync.dma_start(out=we[:], in_=t_we[:])
            wdrow = wt.tile([128, DOUT], bf16)
            nc.sync.dma_start(out=wdrow[:], in_=t_wdrow[:])

            for w in range(NWIN):
                nfe = win.tile([128, tpw * 256], kdt, tag="nfe")
                nc.sync.dma_start(out=nfe[:], in_=t_nfe[w])
                if t_nfe2 is not None:
                    nfe2 = win.tile([128, tpw * 256], vdt, tag="nfe2")
                    nc.sync.dma_start(out=nfe2[:], in_=t_nfe2[w])
                else:
                    nfe2 = nfe
                ohT = win.tile([128, tpw * 128], bf16, tag="ohT")
                nc.scalar.dma_start(out=ohT[:], in_=t_ohT[w])
                ohE = win.tile([128, tpw * 128], bf16, tag="ohE")
                nc.scalar.dma_start(out=ohE[:], in_=t_ohE[w])
                efT = win.tile([64, tpw * 128], bf16, tag="efT")
                nc.sync.dma_start(out=efT[:], in_=t_efT[w])
                nfw = win.tile([128, 256], bf16, tag="nfw")
                nc.sync.dma_start(out=nfw[:], in_=t_nfw[w])

                # qwin = nfw^T @ Wq -> SBUF (borrows a v-pool psum slot)
                ps_qw = psV.tile([128, DOUT], f32, tag="v")
                for i in range(2):
                    nc.tensor.matmul(ps_qw[:], nfw[:, i * 128:(i + 1) * 128],
                                     wq[:, i * DOUT:(i + 1) * DOUT],
                                     start=(i == 0), stop=(i == 1))
                qwin = tb.tile([128, DOUT], bf16, tag="qwin")
                nc.scalar.copy(qwin[:], ps_qw[:])
                if dbg and w == 0:
                    dump("d_qwin", ps_qw[:])

                ps_agg = psA.tile([128, DOUT + 8], f32, tag="agg")
                pend = None  # deferred (wv, agg) of the previous tile
                for t in range(tpw):
                    nfe_t = nfe[:, t * 256:(t + 1) * 256].rearrange(
                        "p (i m) -> p i m", i=2)
                    nfe2_t = nfe2[:, t * 256:(t + 1) * 256].rearrange(
                        "p (i m) -> p i m", i=2)
                    ohT_t = ohT[:, t * 128:(t + 1) * 128]
                    ohE_t = ohE[:, t * 128:(t + 1) * 128]
                    efT_t = efT[:, t * 128:(t + 1) * 128]

                    # kke | qe into one psum tile  [e, hd]
                    ps_kq = psKQ.tile([128, 2 * DOUT], f32, tag="kq")
                    if k_fp8:
                        nc.tensor.matmul(ps_kq[:, DOUT:], nfe_t,
                                         wv3_like_k(wk3), start=True,
                                         stop=False, perf_mode=MM)
                    else:
                        for i in range(2):
                            nc.tensor.matmul(ps_kq[:, DOUT:], nfe_t[:, i, :],
                                             wk3[:, i * DOUT:(i + 1) * DOUT],
                                             start=(i == 0), stop=False)
                    nc.tensor.matmul(ps_kq[:, DOUT:], efT_t, we[:],
                                     start=False, stop=True)
                    nc.tensor.matmul(ps_kq[:, :DOUT], ohT_t, qwin[:],
                                     start=True, stop=True)
                    kke_sb = tb.tile([128, DOUT], bf16, tag="kke_sb")
                    nc.scalar.copy(kke_sb[:], ps_kq[:, DOUT:])

                    # deferred wv+agg of previous tile (gives exp time to land)
                    if pend is not None:
                        emit_wv_agg(nc, pend, ps_agg, H, DH, DOUT)

                    prod = tb.tile([128, DOUT], bf16, tag="prod")
                    nc.vector.tensor_tensor(prod[:], ps_kq[:, :DOUT],
                                            kke_sb[:], mybir.AluOpType.mult)
                    logit = tb.tile([128, 8], f32, tag="logit")
                    nc.vector.tensor_reduce(
                        logit[:], prod[:].rearrange("p (h d) -> p h d", h=H),
                        mybir.AxisListType.X, mybir.AluOpType.add)

                    ps_v = psV.tile([128, DOUT], f32, tag="v")
                    if v_fp8:
                        nc.tensor.matmul(ps_v[:], nfe2_t, wv3[:].rearrange(
                            "p (i n) -> p i n", i=2), start=True, stop=True,
                            perf_mode=MM)
                    else:
                        for i in range(2):
                            nc.tensor.matmul(ps_v[:], nfe2_t[:, i, :],
                                             wv3[:, i * DOUT:(i + 1) * DOUT],
                                             start=(i == 0), stop=(i == 1))

                    wv_on_dve = (t % 8 == 7)
                    if wv_on_dve:
                        v_hd = ps_v
                    else:
                        v_hd = tb.tile([128, DOUT], bf16, tag="v_sb")
                        nc.scalar.copy(v_hd[:], ps_v[:])
                    wv_ext = tb.tile([128, DOUT + 8], bf16, tag="wv_ext")
                    nc.scalar.activation(wv_ext[:, DOUT:], logit[:],
                                         mybir.ActivationFunctionType.Exp)
                    if dbg and w == 0 and t == 0:
                        dump("d_kke", ps_kq[:, DOUT:])
                        dump("d_qe", ps_kq[:, :DOUT])
                        dump("d_prod", prod[:])
                        dump("d_logit", logit[:])
                    pend = (wv_ext, v_hd, ohE_t, t == 0, t == tpw - 1,
                            (dbg and w == 0 and t == 0), wv_on_dve)
                emit_wv_agg(nc, pend, ps_agg, H, DH, DOUT, dump)

                # ---- window finalize ----
                den = tb.tile([128, 8], f32, tag="den")
                nc.vector.tensor_scalar_add(den[:], ps_agg[:, DOUT:], 1e-9)
                recip = tb.tile([128, 8], f32, tag="recip")
                nc.vector.reciprocal(recip[:], den[:])
                u = tb.tile([128, DOUT], bf16, tag="u")
                nc.vector.scalar_tensor_tensor(
                    out=u[:], in0=ps_agg[:, :DOUT], scalar=0.0,
                    in1=wdrow[:], op0=mybir.AluOpType.max,
                    op1=mybir.AluOpType.mult)
                zscr = tb.tile([128, DOUT], bf16, tag="zscr")
                zacc = tb.tile([128, 1], f32, tag="zacc")
                nc.vector.scalar_tensor_tensor(
                    out=zscr[:].rearrange("p (h d) -> p h d", h=H),
                    in0=u[:].rearrange("p (h d) -> p h d", h=H),
                    scalar=0.0,
                    in1=recip[:, :, None].to_broadcast([128, H, DH]),
                    op0=mybir.AluOpType.add, op1=mybir.AluOpType.mult,
                    accum_out=zacc[:])
                if dbg and w == 0:
                    dump("d_den", den[:])
                    dump("d_z", zacc[:])
                ez = tb.tile([128, 1], f32, tag="ez")
                nc.scalar.activation(ez[:], zacc[:],
                                     mybir.ActivationFunctionType.Exp,
                                     scale=-1.0, bias=-float(bd0))
                ez1 = tb.tile([128, 1], f32, tag="ez1")
                nc.vector.tensor_scalar_add(ez1[:], ez[:], 1.0)
                y_sb = tb.tile([128, 1], f32, tag="y_sb")
                nc.vector.reciprocal(y_sb[:], ez1[:])
                nc.sync.dma_start(out=t_y[w], in_=y_sb[:])
    nc.compile()
    return nc


def wv3_like_k(wk3):
    return wk3[:].rearrange("p (i n) -> p i n", i=2)


def emit_wv_agg(nc, pend, ps_agg, H, DH, DOUT, dump=None):
    wv_ext, v_hd, ohE_t, is_first, is_last, do_dump, wv_on_dve = pend
    eng = nc.vector if wv_on_dve else nc.gpsimd
    eng.tensor_tensor(
        wv_ext[:, :DOUT].rearrange("p (h d) -> p h d", h=H),
        wv_ext[:, DOUT:, None].to_broadcast([128, H, DH]),
        v_hd[:].rearrange("p (h d) -> p h d", h=H),
        mybir.AluOpType.mult)
    if do_dump and dump is not None:
        dump("d_wv", wv_ext[:])
    nc.tensor.matmul(ps_agg[:, :DOUT], ohE_t, wv_ext[:, :DOUT],
                     start=is_first, stop=is_last)
    nc.tensor.matmul(ps_agg[:, DOUT:], ohE_t, wv_ext[:, DOUT:],
                     start=is_first, stop=is_last)


# ----------------------------------------------------------------------------
# Entry point
# ----------------------------------------------------------------------------

LAST_RESULTS = None
LAST_NC = None


def prepare(node_features, edge_features, Wq, Wk, Wv, We, Wd, bd, src, dst,
            dbg=False):
    nf = np.asarray(node_features, dtype=np.float32)
    ef = np.asarray(edge_features, dtype=np.float32)
    src = np.asarray(src, dtype=np.int32)
    dst = np.asarray(dst, dtype=np.int32)
    N = nf.shape[0]

    plan = make_plan(src, dst, N, N_CORES, TPW)
    gin = make_global_inputs(np.asarray(Wq, np.float32),
                             np.asarray(Wk, np.float32),
                             np.asarray(Wv, np.float32),
                             np.asarray(We, np.float32),
                             np.asarray(Wd, np.float32))
    ef_sorted_bf = ef[plan["perm"]].astype(BF16)
    nf_bf = nf.astype(BF16)
    nf_e = nf.astype(FP8) if K_FP8 else nf_bf

    nc = build_nc(NWIN=plan["NWIN"], tpw=TPW, LMAX=plan["LMAX"],
                  bd0=float(np.asarray(bd, np.float32).ravel()[0]),
                  k_fp8=K_FP8, v_fp8=V_FP8, dbg=dbg)

    in_maps = []
    core_meta = []
    for c in range(N_CORES):
        cin = make_core_inputs(plan, c, nf_bf, nf_e, ef_sorted_bf)
        m = dict(gin)
        for k in ("nfe", "ohT", "ohE", "efT", "nfw"):
            m[k] = cin[k]
        if "nfe2" in cin:
            m["nfe2"] = cin["nfe2"]
        in_maps.append(m)
        core_meta.append(plan["cores"][c]["wins"])
    return nc, in_maps, core_meta, N


def kernel(node_features, edge_features, Wq, Wk, Wv, We, Wd, bd, src, dst,
           trace=False, dbg=False, n_cores=None):
    from concourse.bass_utils import run_bass_kernel_spmd

    nc, in_maps, core_meta, N = prepare(node_features, edge_features, Wq, Wk,
                                        Wv, We, Wd, bd, src, dst, dbg=dbg)
    ncores = n_cores or N_CORES
    res = run_bass_kernel_spmd(nc, in_maps[:ncores],
                               core_ids=list(range(ncores)), trace=trace)
    global LAST_RESULTS, LAST_NC
    LAST_RESULTS = res
    LAST_NC = nc

    y = np.zeros((N, 1), np.float32)
    for c, wins in enumerate(core_meta[:ncores]):
        yw = res.results[c]["y_out"]
        for w, (wn_lo, wn_hi) in enumerate(wins):
            y[wn_lo:wn_hi, 0] = yw[w, : wn_hi - wn_lo, 0]
    return y


# revision 30
# speedup vs baseline: 2.0945x; 1.0536x over previous
"""GAT message-passing model on 8 Trainium2 NeuronCores.

Strategy: edges sorted by destination on the host; nodes split into 8
contiguous ranges balanced by incoming-edge count (one per core).  Windows of
<=128 contiguous dst nodes with <=TPW*128 edges, padded to TPW tiles of 128
edge slots so all 8 cores run one identical SPMD instruction stream.

The host pre-gathers (pure layout work, no arithmetic) the transposed node
features for every edge slot and window-node block, plus both one-hot
orientations of the edge->node incidence.  The device kernel is then a single
homogeneous window loop with no tables, no fences and no indirect gathers
except the final y scatter:

  per window:  qwin = nfTwin^T @ Wq (PE) -> SBUF (ACT copy)
  per tile:    kke[e,hd] = nfT_e^T Wk + efT^T We   (PE -> psum, ACT copy)
               qe[e,hd]  = ohT^T @ qwin            (PE, same psum tile)
               prod      = qe * kke                (DVE, one PSUM read)
               logit     = reduce_d(prod)          (DVE)
               w         = exp(logit)              (ACT, into wv_ext[:,512:])
               v[e,hd]   = nfT_e^T @ Wv            (PE; ACT copy to SBUF)
               wv        = w (bcast) * v           (POOL; every 8th on DVE)
               agg, den += ohE^T @ [wv | w]        (PE segment-sum, deferred
                                                    one tile for pipelining)
  finalize:    den+eps, recip (DVE); u=relu(agg)*wd (DVE STT);
               z=sum(u*recip) (DVE STT+accum); y=1/(1+exp(-(z+bd)))
               (ACT exp + DVE); dense per-window y DMA, host scatters.
"""

import numpy as np
import ml_dtypes

import concourse.bass as bass
import concourse.bacc as bacc
import concourse.mybir as mybir
import concourse.tile as tile

BF16 = ml_dtypes.bfloat16
FP8 = ml_dtypes.float8_e4m3

H, DH = 8, 64
DIN, DE = 256, 64
DOUT = H * DH  # 512
N_CORES = 8
TPW = 8  # edge tiles per window
K_FP8 = False
V_FP8 = False


# ----------------------------------------------------------------------------
# Host-side planning (layout only -- no arithmetic on features/weights)
# ----------------------------------------------------------------------------

def make_plan(src, dst, n_nodes, n_cores, tpw):
    E = src.shape[0]
    perm = np.argsort(dst, kind="stable")
    s_src = src[perm]
    s_dst = dst[perm]
    deg = np.bincount(dst, minlength=n_nodes)
    cum = np.concatenate([[0], np.cumsum(deg)])

    cuts = [0]
    for c in range(1, n_cores):
        target = c * E / n_cores
        n = int(np.searchsorted(cum, target))
        n = max(cuts[-1] + 1, min(n, n_nodes - (n_cores - c)))
        cuts.append(n)
    cuts.append(n_nodes)

    cores = []
    for c in range(n_cores):
        nlo, nhi = cuts[c], cuts[c + 1]
        wins = []
        n = nlo
        while n < nhi:
            n2 = n
            edges = 0
            while n2 < nhi and (n2 - n) < 128:
                if edges + deg[n2] > tpw * 128:
                    break
                edges += deg[n2]
                n2 += 1
            assert n2 > n, f"node {n} degree {deg[n]} > {tpw*128}"
            wins.append((n, n2))
            n = n2
        cores.append(dict(nlo=nlo, nhi=nhi, wins=wins))

    NWIN = max(len(c["wins"]) for c in cores)
    LMAX = max(c["nhi"] - c["nlo"] for c in cores)
    return dict(cores=cores, NWIN=NWIN, LMAX=LMAX, TPW=tpw,
                s_src=s_src, s_dst=s_dst, perm=perm, cum=cum)


def make_core_inputs(plan, core_idx, nf_bf, nf_e_dt, ef_sorted_bf):
    """Per-core pre-gathered tensors.  nf_e_dt: nf in the k-path dtype."""
    tpw = plan["TPW"]
    NWIN = plan["NWIN"]
    LMAX = plan["LMAX"]
    core = plan["cores"][core_idx]
    s_src, s_dst, cum = plan["s_src"], plan["s_dst"], plan["cum"]
    nlo = core["nlo"]
    L = core["nhi"] - nlo
    trash = LMAX

    nfe = np.zeros((NWIN, 128, tpw, 2, 128), nf_e_dt.dtype)
    ohT = np.zeros((NWIN, 128, tpw * 128), BF16)
    ohE = np.zeros((NWIN, 128, tpw * 128), BF16)
    efT = np.zeros((NWIN, 64, tpw * 128), BF16)
    nfw = np.zeros((NWIN, 128, 2, 128), BF16)
    wnodes = np.full((NWIN, 128, 1), trash, np.int32)

    for w, (wn_lo, wn_hi) in enumerate(core["wins"]):
        e0, e1 = cum[wn_lo], cum[wn_hi]
        cnt = e1 - e0
        Lw = wn_hi - wn_lo
        wnodes[w, :Lw, 0] = np.arange(wn_lo, wn_hi) - nlo
        sl = np.arange(cnt)
        t_idx = sl // 128
        p_idx = sl % 128
        dl = s_dst[e0:e1] - wn_lo
        # transposed gathered node features: nfe[w, p, t, i, e] = nf[src, i*128+p]
        blk = nf_e_dt[s_src[e0:e1]].reshape(cnt, 2, 128)  # [slot, i, p]
        nfe[w][:, t_idx, :, p_idx] = blk.transpose(0, 2, 1)
        # one-hots
        ohT[w][dl, t_idx * 128 + p_idx] = 1.0
        ohE[w][p_idx, t_idx * 128 + dl] = 1.0
        # transposed edge features
        efT[w][:, t_idx * 128 + p_idx] = ef_sorted_bf[e0:e1].T
        # transposed window-node features for q
        nblk = nf_bf[wn_lo:wn_hi].reshape(Lw, 2, 128)  # [nl, i, p]
        nfw[w][:, :, :Lw] = nblk.transpose(2, 1, 0)
    out = dict(nfe=nfe.reshape(NWIN, 128, tpw * 256),
               ohT=ohT, ohE=ohE, efT=efT,
               nfw=nfw.reshape(NWIN, 128, 256),
               wnodes=wnodes, L=L, nlo=nlo)
    if K_FP8 != V_FP8:
        out["nfe2"] = out["nfe"].astype(FP8 if V_FP8 else BF16)
    return out


def make_global_inputs(Wq, Wk, Wv, We, Wd):
    scale = 1.0 / np.sqrt(DH)
    # DoubleRow K-packing: w3[p, i, n] = W[i*128+p, n]
    wk3 = np.zeros((128, 2, DOUT), FP8 if K_FP8 else BF16)
    wv3 = np.zeros((128, 2, DOUT), FP8 if V_FP8 else BF16)
    for i in range(2):
        wk3[:, i, :] = Wk[i * 128:(i + 1) * 128].astype(wk3.dtype)
        wv3[:, i, :] = Wv[i * 128:(i + 1) * 128].astype(wv3.dtype)
    # wq[p, c*512+n] = (Wq*scale)[c*128+p, n]
    wq = np.concatenate([(Wq * scale)[:128], (Wq * scale)[128:256]],
                        axis=1).astype(BF16)
    we = We.astype(BF16)                       # [64, 512]
    wdrow = np.tile(Wd.reshape(1, DOUT), (128, 1)).astype(BF16)
    return dict(wk3=wk3.reshape(128, 2 * DOUT), wv3=wv3.reshape(128, 2 * DOUT),
                wq=wq, we=we, wdrow=wdrow)


# ----------------------------------------------------------------------------
# Device kernel emission (identical instruction stream on every core)
# ----------------------------------------------------------------------------

def build_nc(NWIN, tpw, LMAX, bd0, k_fp8, v_fp8, dbg=False):
    dt = mybir.dt
    bf16, f32, i32 = dt.bfloat16, dt.float32, dt.int32
    kdt = dt.float8e4 if k_fp8 else dt.bfloat16
    vdt = dt.float8e4 if v_fp8 else dt.bfloat16
    YROWS = LMAX + 128

    nc = bacc.Bacc("TRN2", target_bir_lowering=False, debug=False)

    t_nfe = nc.dram_tensor("nfe", [NWIN, 128, tpw * 256], kdt, kind="ExternalInput")
    t_nfe2 = (nc.dram_tensor("nfe2", [NWIN, 128, tpw * 256], vdt,
                             kind="ExternalInput") if k_fp8 != v_fp8 else None)
    t_ohT = nc.dram_tensor("ohT", [NWIN, 128, tpw * 128], bf16, kind="ExternalInput")
    t_ohE = nc.dram_tensor("ohE", [NWIN, 128, tpw * 128], bf16, kind="ExternalInput")
    t_efT = nc.dram_tensor("efT", [NWIN, 64, tpw * 128], bf16, kind="ExternalInput")
    t_nfw = nc.dram_tensor("nfw", [NWIN, 128, 256], bf16, kind="ExternalInput")
    t_wk3 = nc.dram_tensor("wk3", [128, 2 * DOUT], kdt, kind="ExternalInput")
    t_wv3 = nc.dram_tensor("wv3", [128, 2 * DOUT], vdt, kind="ExternalInput")
    t_wq = nc.dram_tensor("wq", [128, 2 * DOUT], bf16, kind="ExternalInput")
    t_we = nc.dram_tensor("we", [64, DOUT], bf16, kind="ExternalInput")
    t_wdrow = nc.dram_tensor("wdrow", [128, DOUT], bf16, kind="ExternalInput")

    t_y = nc.dram_tensor("y_out", [NWIN, 128, 1], f32, kind="ExternalOutput")
    t_dbg = {}
    if dbg:
        for nm, shp in [("d_kke", [128, 512]), ("d_qe", [128, 512]),
                        ("d_prod", [128, 512]), ("d_logit", [128, 8]),
                        ("d_wv", [128, 520]), ("d_qwin", [128, 512]),
                        ("d_den", [128, 8]), ("d_scr", [128, 512]),
                        ("d_z", [128, 1])]:
            t_dbg[nm] = nc.dram_tensor(nm, shp, f32, kind="ExternalOutput")

    def dump(nm, ap):
        if dbg:
            nc.gpsimd.dma_start(out=t_dbg[nm][: ap.shape[0]], in_=ap)

    MM = mybir.MatmulPerfMode.DoubleRow

    with tile.TileContext(nc, pool_alloc_mode="queue") as tc:
        with tc.tile_pool(name="wt", bufs=1) as wt, \
             tc.tile_pool(name="win", bufs=4) as win, \
             tc.tile_pool(name="tb", bufs=5) as tb, \
             tc.tile_pool(name="psKQ", bufs=2, space="PSUM") as psKQ, \
             tc.tile_pool(name="psV", bufs=2, space="PSUM") as psV, \
             tc.tile_pool(name="psA", bufs=1, space="PSUM") as psA:
            wk3 = wt.tile([128, 2 * DOUT], kdt)
            nc.sync.dma_start(out=wk3[:], in_=t_wk3[:])
            wv3 = wt.tile([128, 2 * DOUT], vdt)
            nc.sync.dma_start(out=wv3[:], in_=t_wv3[:])
            wq = wt.tile([128, 2 * DOUT], bf16)
            nc.sync.dma_start(out=wq[:], in_=t_wq[:])
            we = wt.tile([64, DOUT], bf16)
            nc.sync.dma_start(out=we[:], in_=t_we[:])
            wdrow = wt.tile([128, DOUT], bf16)
            nc.sync.dma_start(out=wdrow[:], in_=t_wdrow[:])

            for w in range(NWIN):
                nfe = win.tile([128, tpw * 256], kdt, tag="nfe")
                nc.sync.dma_start(out=nfe[:], in_=t_nfe[w])
                if t_nfe2 is not None:
                    nfe2 = win.tile([128, tpw * 256], vdt, tag="nfe2")
                    nc.sync.dma_start(out=nfe2[:], in_=t_nfe2[w])
                else:
                    nfe2 = nfe
                ohT = win.tile([128, tpw * 128], bf16, tag="ohT")
                nc.scalar.dma_start(out=ohT[:], in_=t_ohT[w])
                ohE = win.tile([128, tpw * 128], bf16, tag="ohE")
                nc.scalar.dma_start(out=ohE[:], in_=t_ohE[w])
                efT = win.tile([64, tpw * 128], bf16, tag="efT")
                nc.sync.dma_start(out=efT[:], in_=t_efT[w])
                nfw = win.tile([128, 256], bf16, tag="nfw")
                nc.sync.dma_start(out=nfw[:], in_=t_nfw[w])

                # qwin = nfw^T @ Wq -> SBUF (borrows a v-pool psum slot)
                ps_qw = psV.tile([128, DOUT], f32, tag="v")
                for i in range(2):
                    nc.tensor.matmul(ps_qw[:], nfw[:, i * 128:(i + 1) * 128],
                                     wq[:, i * DOUT:(i + 1) * DOUT],
                                     start=(i == 0), stop=(i == 1))
                qwin = tb.tile([128, DOUT], bf16, tag="qwin")
                nc.scalar.copy(qwin[:], ps_qw[:])
                if dbg and w == 0:
                    dump("d_qwin", ps_qw[:])

                ps_agg = psA.tile([128, DOUT + 8], f32, tag="agg")
                pend = None  # deferred (wv, agg) of the previous tile
                for t in range(tpw):
                    nfe_t = nfe[:, t * 256:(t + 1) * 256].rearrange(
                        "p (i m) -> p i m", i=2)
                    nfe2_t = nfe2[:, t * 256:(t + 1) * 256].rearrange(
                        "p (i m) -> p i m", i=2)
                    ohT_t = ohT[:, t * 128:(t + 1) * 128]
                    ohE_t = ohE[:, t * 128:(t + 1) * 128]
                    efT_t = efT[:, t * 128:(t + 1) * 128]

                    # kke | qe into one psum tile  [e, hd]
                    ps_kq = psKQ.tile([128, 2 * DOUT], f32, tag="kq")
                    if k_fp8:
                        nc.tensor.matmul(ps_kq[:, DOUT:], nfe_t,
                                         wv3_like_k(wk3), start=True,
                                         stop=False, perf_mode=MM)
                    else:
                        for i in range(2):
                            nc.tensor.matmul(ps_kq[:, DOUT:], nfe_t[:, i, :],
                                             wk3[:, i * DOUT:(i + 1) * DOUT],
                                             start=(i == 0), stop=False)
                    nc.tensor.matmul(ps_kq[:, DOUT:], efT_t, we[:],
                                     start=False, stop=True)
                    nc.tensor.matmul(ps_kq[:, :DOUT], ohT_t, qwin[:],
                                     start=True, stop=True)
                    kke_sb = tb.tile([128, DOUT], bf16, tag="kke_sb")
                    nc.scalar.copy(kke_sb[:], ps_kq[:, DOUT:])

                    # deferred wv+agg of previous tile (gives exp time to land)
                    if pend is not None:
                        emit_wv_agg(nc, pend, ps_agg, H, DH, DOUT)

                    prod = tb.tile([128, DOUT], bf16, tag="prod")
                    nc.vector.tensor_tensor(prod[:], ps_kq[:, :DOUT],
                                            kke_sb[:], mybir.AluOpType.mult)
                    logit = tb.tile([128, 8], f32, tag="logit")
                    nc.vector.tensor_reduce(
                        logit[:], prod[:].rearrange("p (h d) -> p h d", h=H),
                        mybir.AxisListType.X, mybir.AluOpType.add)

                    ps_v = psV.tile([128, DOUT], f32, tag="v")
                    if v_fp8:
                        nc.tensor.matmul(ps_v[:], nfe2_t, wv3[:].rearrange(
                            "p (i n) -> p i n", i=2), start=True, stop=True,
                            perf_mode=MM)
                    else:
                        for i in range(2):
                            nc.tensor.matmul(ps_v[:], nfe2_t[:, i, :],
                                             wv3[:, i * DOUT:(i + 1) * DOUT],
                                             start=(i == 0), stop=(i == 1))

                    wv_on_dve = False
                    if wv_on_dve:
                        v_hd = ps_v
                    else:
                        v_hd = tb.tile([128, DOUT], bf16, tag="v_sb")
                        nc.scalar.copy(v_hd[:], ps_v[:])
                    wv_ext = tb.tile([128, DOUT + 8], bf16, tag="wv_ext")
                    nc.scalar.activation(wv_ext[:, DOUT:], logit[:],
                                         mybir.ActivationFunctionType.Exp)
                    if dbg and w == 0 and t == 0:
                        dump("d_kke", ps_kq[:, DOUT:])
                        dump("d_qe", ps_kq[:, :DOUT])
                        dump("d_prod", prod[:])
                        dump("d_logit", logit[:])
                    pend = (wv_ext, v_hd, ohE_t, t == 0, t == tpw - 1,
                            (dbg and w == 0 and t == 0), wv_on_dve)
                emit_wv_agg(nc, pend, ps_agg, H, DH, DOUT, dump)

                # ---- window finalize ----
                den = tb.tile([128, 8], f32, tag="den")
                nc.vector.tensor_scalar_add(den[:], ps_agg[:, DOUT:], 1e-9)
                recip = tb.tile([128, 8], f32, tag="recip")
                nc.vector.reciprocal(recip[:], den[:])
                u = tb.tile([128, DOUT], bf16, tag="u")
                nc.vector.scalar_tensor_tensor(
                    out=u[:], in0=ps_agg[:, :DOUT], scalar=0.0,
                    in1=wdrow[:], op0=mybir.AluOpType.max,
                    op1=mybir.AluOpType.mult)
                zscr = tb.tile([128, DOUT], bf16, tag="zscr")
                zacc = tb.tile([128, 1], f32, tag="zacc")
                nc.vector.scalar_tensor_tensor(
                    out=zscr[:].rearrange("p (h d) -> p h d", h=H),
                    in0=u[:].rearrange("p (h d) -> p h d", h=H),
                    scalar=0.0,
                    in1=recip[:, :, None].to_broadcast([128, H, DH]),
                    op0=mybir.AluOpType.add, op1=mybir.AluOpType.mult,
                    accum_out=zacc[:])
                if dbg and w == 0:
                    dump("d_den", den[:])
                    dump("d_z", zacc[:])
                ez = tb.tile([128, 1], f32, tag="ez")
                nc.scalar.activation(ez[:], zacc[:],
                                     mybir.ActivationFunctionType.Exp,
                                     scale=-1.0, bias=-float(bd0))
                ez1 = tb.tile([128, 1], f32, tag="ez1")
                nc.vector.tensor_scalar_add(ez1[:], ez[:], 1.0)
                y_sb = tb.tile([128, 1], f32, tag="y_sb")
                nc.vector.reciprocal(y_sb[:], ez1[:])
                nc.sync.dma_start(out=t_y[w], in_=y_sb[:])
    nc.compile()
    return nc


def wv3_like_k(wk3):
    return wk3[:].rearrange("p (i n) -> p i n", i=2)


def emit_wv_agg(nc, pend, ps_agg, H, DH, DOUT, dump=None):
    wv_ext, v_hd, ohE_t, is_first, is_last, do_dump, wv_on_dve = pend
    eng = nc.vector if wv_on_dve else nc.gpsimd
    eng.tensor_tensor(
        wv_ext[:, :DOUT].rearrange("p (h d) -> p h d", h=H),
        wv_ext[:, DOUT:, None].to_broadcast([128, H, DH]),
        v_hd[:].rearrange("p (h d) -> p h d", h=H),
        mybir.AluOpType.mult)
    if do_dump and dump is not None:
        dump("d_wv", wv_ext[:])
    nc.tensor.matmul(ps_agg[:, :DOUT], ohE_t, wv_ext[:, :DOUT],
                     start=is_first, stop=is_last)
    nc.tensor.matmul(ps_agg[:, DOUT:], ohE_t, wv_ext[:, DOUT:],
                     start=is_first, stop=is_last)


# ----------------------------------------------------------------------------
# Entry point
# ----------------------------------------------------------------------------

LAST_RESULTS = None
LAST_NC = None


def prepare(node_features, edge_features, Wq, Wk, Wv, We, Wd, bd, src, dst,
            dbg=False):
    nf = np.asarray(node_features, dtype=np.float32)
    ef = np.asarray(edge_features, dtype=np.float32)
    src = np.asarray(src, dtype=np.int32)
    dst = np.asarray(dst, dtype=np.int32)
    N = nf.shape[0]

    plan = make_plan(src, dst, N, N_CORES, TPW)
    gin = make_global_inputs(np.asarray(Wq, np.float32),
                             np.asarray(Wk, np.float32),
                             np.asarray(Wv, np.float32),
                             np.asarray(We, np.float32),
                             np.asarray(Wd, np.float32))
    ef_sorted_bf = ef[plan["perm"]].astype(BF16)
    nf_bf = nf.astype(BF16)
    nf_e = nf.astype(FP8) if K_FP8 else nf_bf

    nc = build_nc(NWIN=plan["NWIN"], tpw=TPW, LMAX=plan["LMAX"],
                  bd0=float(np.asarray(bd, np.float32).ravel()[0]),
                  k_fp8=K_FP8, v_fp8=V_FP8, dbg=dbg)

    in_maps = []
    core_meta = []
    for c in range(N_CORES):
        cin = make_core_inputs(plan, c, nf_bf, nf_e, ef_sorted_bf)
        m = dict(gin)
        for k in ("nfe", "ohT", "ohE", "efT", "nfw"):
            m[k] = cin[k]
        if "nfe2" in cin:
            m["nfe2"] = cin["nfe2"]
        in_maps.append(m)
        core_meta.append(plan["cores"][c]["wins"])
    return nc, in_maps, core_meta, N


def kernel(node_features, edge_features, Wq, Wk, Wv, We, Wd, bd, src, dst,
           trace=False, dbg=False, n_cores=None):
    from concourse.bass_utils import run_bass_kernel_spmd

    nc, in_maps, core_meta, N = prepare(node_features, edge_features, Wq, Wk,
                                        Wv, We, Wd, bd, src, dst, dbg=dbg)
    ncores = n_cores or N_CORES
    res = run_bass_kernel_spmd(nc, in_maps[:ncores],
                               core_ids=list(range(ncores)), trace=trace)
    global LAST_RESULTS, LAST_NC
    LAST_RESULTS = res
    LAST_NC = nc

    y = np.zeros((N, 1), np.float32)
    for c, wins in enumerate(core_meta[:ncores]):
        yw = res.results[c]["y_out"]
        for w, (wn_lo, wn_hi) in enumerate(wins):
            y[wn_lo:wn_hi, 0] = yw[w, : wn_hi - wn_lo, 0]
    return y
